# revision 3
# baseline (speedup 1.0000x reference)
"""Self-contained Trainium2 (Bass/Tile) kernel: single-head causal attention.

Problem: embeddings [4,4096,1024] f32; Wq/Wk/Wv [1024,1024] f32 (torch Linear
layout [out,in]).  out = softmax(causal(QK^T)/sqrt(D)) @ V, computed per batch.

Distribution (v3): 8 NeuronCores, one SPMD program, context-parallel split.
Core c handles batch c//2; the two cores of a batch pair split the KEY axis by
interleaved 128-row k-tiles (even core: true tiles 0,2,4,...; odd: 1,3,5,...).
Each core processes ALL 4096 query rows against its 2048 k-rows and emits
unnormalized partial attention (sum of exp-weights times V) plus the partial
softmax denominators; the host unshard step adds the pair's partials and
divides.  This is the standard sequence/context-parallel attention combine.

Per-core work: slot j = query chunk j (256 rows) needs exactly j+1 of this
core's k-tiles (perfect causal balance; only each slot's last tile is
diagonal-masked, via a per-core mask table input).  V is projected only for
this core's k-rows (no duplicate work in the pair) and stays resident in
SBUF.  scores = emb_q M emb_k^T with M = Wq^T Wk folded on the K side:
KP = M @ embk^T, so neither Q nor K is ever materialized.

Host-side staging: transpose + bf16-cast, k-tile gather for embk, mask table,
and the final pair combine (add partials, divide by summed denominator).
All matmuls (M, V, KP, scores, AV) and the exp run on device in bf16 with f32
accumulation.
"""

import math
import os
import sys
import types

import numpy as np
import ml_dtypes

B, S, D = 4, 4096, 1024
NCORES = 8
NSLOT = 16
CHUNK = 256          # q rows per slot
KHALF = S // 2       # k rows owned per core
INV_SQRT_D = 1.0 / math.sqrt(D)
BF16 = ml_dtypes.bfloat16


# ---------------------------------------------------------------------------
# Environment patches (compiler workarounds + profiling hook shim)
# ---------------------------------------------------------------------------

def _install_patches():
    import json as _json
    import concourse.bass as bass

    if not getattr(bass.Bass, "_mw_patched", False):
        _orig_to_json = bass.Bass.to_json_bytes

        def to_json_bytes(self):
            # This walrus build rejects any instruction carrying more than one
            # sync wait ("Too many sync wait commands").  Split extra waits
            # onto single-wait NoOps inserted just before the instruction (the
            # engine executes them in order, so semantics are unchanged).
            raw = _orig_to_json(self)
            m = _json.loads(raw)
            ctr = 0
            changed = False
            for fn in m.get("functions", []):
                for bb in fn.get("blocks", []):
                    out = []
                    for inst in bb.get("instructions", []):
                        si = inst.get("sync_info")
                        if si:
                            waits = si.get("on_wait") or []
                            if len(waits) > 1:
                                changed = True
                                for w in waits[:-1]:
                                    ctr += 1
                                    out.append({
                                        "debug": inst.get("debug", 0),
                                        "engine": inst["engine"],
                                        "ins": [],
                                        "outs": [],
                                        "name": f"I-mw{ctr}",
                                        "opcode": "NoOp",
                                        "text_hint": "mwsplit",
                                        "sync_info": {"on_wait": [w],
                                                      "on_update": []},
                                    })
                                si["on_wait"] = [waits[-1]]
                        out.append(inst)
                    bb["instructions"] = out
            if not changed:
                return raw
            return _json.dumps(m).encode()

        bass.Bass.to_json_bytes = to_json_bytes
        bass.Bass._mw_patched = True

    # Don't upload NEFF/trace artifacts anywhere; keep them local.
    import concourse.bass_utils as bu
    bu.upload_artifacts = lambda tmpdir: tmpdir


def _install_ntff_hook() -> bool:
    """Register the axon NTFF profiling hook (missing module in this image)."""
    try:
        import antenv.axon_hooks  # noqa: F401
        return True
    except ImportError:
        pass
    try:
        mod = types.ModuleType("antenv.axon_hooks")
        state = {"hook": None}
        mod.set_axon_ntff_profile_hook = lambda h: state.__setitem__("hook", h)
        mod.get_axon_ntff_profile_hook = lambda: state["hook"]
        sys.modules["antenv.axon_hooks"] = mod
        import antenv
        antenv.axon_hooks = mod
        from trn_agent_boot.trn_boot import _ntff_profile_via_ctypes
        mod.set_axon_ntff_profile_hook(
            _ntff_profile_via_ctypes("/opt/axon/libaxon_pjrt.so"))
        return True
    except Exception:
        return False


# ---------------------------------------------------------------------------
# Graph
# ---------------------------------------------------------------------------

def _build_graph():
    import concourse.bass as bass
    import concourse.mybir as mybir
    import concourse.tile as tile

    f32 = mybir.dt.float32
    bf16 = mybir.dt.bfloat16
    Exp = mybir.ActivationFunctionType.Exp

    nc = bass.Bass("TRN2", debug=False, num_devices=NCORES)

    embT_in = nc.dram_tensor("embT", [D, S], bf16, kind="ExternalInput")
    embkT_in = nc.dram_tensor("embkT", [D, KHALF], bf16, kind="ExternalInput")
    wq_in = nc.dram_tensor("wqn", [D, D], bf16, kind="ExternalInput")
    wk_in = nc.dram_tensor("wkn", [D, D], bf16, kind="ExternalInput")
    wvT_in = nc.dram_tensor("wvT", [D, D], bf16, kind="ExternalInput")
    masks_in = nc.dram_tensor("masks", [NSLOT, 128, CHUNK], bf16,
                              kind="ExternalInput")
    oav_d = nc.dram_tensor("oav", [S, D], f32, kind="ExternalOutput")
    ol_d = nc.dram_tensor("ol", [NSLOT, 128, 2], f32, kind="ExternalOutput")

    with tile.TileContext(nc) as tc:
        with (
            tc.tile_pool(name="wsb", bufs=1) as wsb,          # weights resident
            tc.tile_pool(name="eksb", bufs=1) as eksb,        # embk^T resident
            tc.tile_pool(name="vsb", bufs=1) as vsb,          # V resident
            tc.tile_pool(name="eqs", bufs=16) as eqs,         # embT q-side stream
            tc.tile_pool(name="mks", bufs=4) as mks,          # mask stream
            tc.tile_pool(name="wts", bufs=6) as wts,          # exp weights
            tc.tile_pool(name="outs", bufs=3) as outs,        # av out stage
            tc.tile_pool(name="smalls", bufs=4) as smalls,
            tc.tile_pool(name="pmm", bufs=2, space="PSUM") as pmm,
            tc.tile_pool(name="ps", bufs=2, space="PSUM") as ps_pool,
            tc.tile_pool(name="pl", bufs=1, space="PSUM") as pl_pool,
        ):
            # constants
            ones = smalls.tile([128, 1], bf16, name="ones", tag="ones")
            nc.gpsimd.memset(ones[:], 1.0)

            # resident weight tiles; DMA order = need order (M first)
            wk_n, wq_n, wv_t = [], [], []
            for dc in range(8):
                t = wsb.tile([128, D], bf16, name=f"wk{dc}", tag=f"wk{dc}")
                nc.sync.dma_start(t[:], wk_in[dc * 128:(dc + 1) * 128, :])
                wk_n.append(t)
                t = wsb.tile([128, D], bf16, name=f"wq{dc}", tag=f"wq{dc}")
                nc.sync.dma_start(t[:], wq_in[dc * 128:(dc + 1) * 128, :])
                wq_n.append(t)
            embk_sb = []
            for dc in range(8):
                t = eksb.tile([128, KHALF], bf16, name=f"ek{dc}",
                              tag=f"ek{dc}")
                nc.sync.dma_start(t[:], embkT_in[dc * 128:(dc + 1) * 128, :])
                embk_sb.append(t)
            for dc in range(8):
                t = wsb.tile([128, D], bf16, name=f"wv{dc}", tag=f"wv{dc}")
                nc.sync.dma_start(t[:], wvT_in[dc * 128:(dc + 1) * 128, :])
                wv_t.append(t)

            # ---------------- MT = Wk^T @ Wq  [d', d] ----------------
            # scores = emb_q M emb_k^T with M[d,d'] = sum_e Wq[e,d] Wk[e,d'];
            # we materialize M^T (tiles [128d', 1024d]) as the lhsT source for
            # the K-side fold below.
            mt_sb = []
            for ac in range(8):
                psum = pmm.tile([128, 1024], f32, name=f"pm{ac}", tag="mm")
                for bb in range(2):
                    for ec in range(8):
                        nc.tensor.matmul(
                            psum[:, bb * 512:(bb + 1) * 512],
                            wk_n[ec][:, ac * 128:(ac + 1) * 128],
                            wq_n[ec][:, bb * 512:(bb + 1) * 512],
                            start=(ec == 0), stop=(ec == 7))
                t = wsb.tile([128, D], bf16, name=f"mt{ac}", tag=f"mt{ac}")
                nc.scalar.copy(t[:], psum[:])
                mt_sb.append(t)

            # ---------------- V projection (my k rows), SBUF resident -------
            v_sb = []
            for sr in range(16):
                psum = pmm.tile([128, 1024], f32, name=f"pv{sr}", tag="mm")
                col = sr * 128
                for eb in range(2):
                    for dc in range(8):
                        nc.tensor.matmul(
                            psum[:, eb * 512:(eb + 1) * 512],
                            embk_sb[dc][:, col:col + 128],
                            wv_t[dc][:, eb * 512:(eb + 1) * 512],
                            start=(dc == 0), stop=(dc == 7))
                t = vsb.tile([128, 1024], bf16, name=f"v{sr}", tag=f"v{sr}")
                nc.scalar.copy(t[:], psum[:])
                v_sb.append(t)

            # ---------------- KP = M @ embk^T  [d, k] (scores lhsT) ---------
            # KP[d,k] = sum_d' MT[d',d] embk^T[d',k].  16 half-tiles
            # [128d, 1024k]; kh=0 (k tiles 0..7) first so early slots can
            # start while kh=1 computes.  Aliases the dead wk and wv buffers.
            kp_sb = [None] * 16
            for kh in range(2):
                for dc in range(8):
                    psum = pmm.tile([128, 1024], f32, name=f"pk{kh}_{dc}",
                                    tag="mm")
                    for kb in range(2):
                        koff = kh * 1024 + kb * 512
                        for ec in range(8):
                            nc.tensor.matmul(
                                psum[:, kb * 512:(kb + 1) * 512],
                                mt_sb[ec][:, dc * 128:(dc + 1) * 128],
                                embk_sb[ec][:, koff:koff + 512],
                                start=(ec == 0), stop=(ec == 7))
                    alias = f"wk{dc}" if kh == 0 else f"wv{dc}"
                    t = wsb.tile([128, 1024], bf16, name=f"kp{kh}_{dc}",
                                 tag=alias)
                    nc.scalar.copy(t[:], psum[:])
                    kp_sb[dc * 2 + kh] = t

            # ---------------- attention ----------------
            # slot j = query chunk j (rows 256j..256j+255); k tiles 0..j of
            # this core's half; tile kt lives in kp_sb[dc*2 + kt//8] at column
            # block (kt%8)*128 and v_sb[kt].  Only kt==j is diagonal-masked.
            for j in range(NSLOT):
                eq = []
                for dc in range(8):
                    t = eqs.tile([128, CHUNK], bf16, name=f"eq{j}_{dc}",
                                 tag="eqs")
                    nc.sync.dma_start(
                        t[:], embT_in[dc * 128:(dc + 1) * 128,
                                      j * CHUNK:(j + 1) * CHUNK])
                    eq.append(t)
                mkt = mks.tile([128, CHUNK], bf16, name=f"mk{j}", tag="mks")
                nc.sync.dma_start(mkt[:], masks_in[j, :, :])

                l_ps = [pl_pool.tile([128, 1], f32, name=f"l{j}_{qs}",
                                     tag=f"l{qs}") for qs in range(2)]
                av = [pmm.tile([128, 1024], f32, name=f"av{j}_{qs}", tag="mm")
                      for qs in range(2)]

                # Software-pipelined by one k-tile: scores(kt+1) issue before
                # AV(kt) so the tensor engine hides the scalar exp latency.
                def emit_av(wt, kt):
                    first, last = kt == 0, kt == j
                    vt = v_sb[kt]
                    for qs in range(2):
                        wslice = wt[:, qs * 128:(qs + 1) * 128]
                        nc.tensor.matmul(l_ps[qs][:], wslice, ones[:],
                                         start=first, stop=last)
                        for eb in range(2):
                            nc.tensor.matmul(
                                av[qs][:, eb * 512:(eb + 1) * 512], wslice,
                                vt[:, eb * 512:(eb + 1) * 512],
                                start=first, stop=last)

                pend = None
                for kt in range(j + 1):
                    s_ps = ps_pool.tile([128, CHUNK], f32, name=f"s{j}_{kt}",
                                        tag="s")
                    half = kt // 8
                    kcol = (kt % 8) * 128
                    for dc in range(8):
                        nc.tensor.matmul(
                            s_ps[:], kp_sb[dc * 2 + half][:, kcol:kcol + 128],
                            eq[dc][:], start=(dc == 0), stop=(dc == 7))

                    wt = wts.tile([128, CHUNK], bf16, name=f"w{j}_{kt}",
                                  tag="wts")
                    nc.scalar.activation(wt[:], s_ps[:], Exp, bias=0.0,
                                         scale=INV_SQRT_D)
                    if kt == j:
                        nc.vector.tensor_mul(wt[:], wt[:], mkt[:])

                    if pend is not None:
                        emit_av(*pend)
                    pend = (wt, kt)
                emit_av(*pend)

                # unnormalized partials; the host divides by the pair-summed
                # denominator.  (DMA can't source PSUM, so stage via SBUF.)
                l_sb = smalls.tile([128, 2], f32, name=f"ls{j}", tag="lst")
                for qs in range(2):
                    nc.vector.tensor_copy(l_sb[:, qs:qs + 1], l_ps[qs][:])
                nc.gpsimd.dma_start(ol_d[j, :, :], l_sb[:])
                for qs in range(2):
                    o_sb = outs.tile([128, 1024], f32, name=f"o{j}_{qs}",
                                     tag="outs")
                    nc.vector.tensor_copy(o_sb[:], av[qs][:])
                    row = (j * 2 + qs) * 128
                    nc.gpsimd.dma_start(oav_d[row:row + 128, :], o_sb[:])

    return nc


_CACHED = {}


def _get_graph():
    if "nc" not in _CACHED:
        _install_patches()
        _CACHED["nc"] = _build_graph()
    return _CACHED["nc"]


# ---------------------------------------------------------------------------
# Host-side staging
# ---------------------------------------------------------------------------

def _masks(parity):
    m = np.zeros((NSLOT, 128, CHUNK), dtype=np.float32)
    for j in range(NSLOT):
        p = np.arange(128)[:, None]
        x = np.arange(CHUNK)[None, :]
        m[j] = ((j * CHUNK + x) >= ((2 * j + parity) * 128 + p))
    return m.astype(BF16)


def kernel(embeddings, Wq, Wk, Wv):
    embeddings = np.asarray(embeddings, dtype=np.float32)
    Wq = np.asarray(Wq, dtype=np.float32)
    Wk = np.asarray(Wk, dtype=np.float32)
    Wv = np.asarray(Wv, dtype=np.float32)

    nc = _get_graph()
    from concourse.bass_utils import run_bass_kernel_spmd

    wqn = Wq.astype(BF16)
    wkn = Wk.astype(BF16)
    wvT = np.ascontiguousarray(Wv.T).astype(BF16)
    masks_by_par = [_masks(0), _masks(1)]

    in_maps = []
    for c in range(NCORES):
        b, par = divmod(c, 2)
        emb_b = embeddings[b]
        embT = np.ascontiguousarray(emb_b.T).astype(BF16)
        # my k rows: interleaved 128-row tiles (2t+par for t in 0..15)
        embk = np.concatenate(
            [emb_b[(2 * t + par) * 128:(2 * t + par) * 128 + 128]
             for t in range(16)], axis=0)
        embkT = np.ascontiguousarray(embk.T).astype(BF16)
        in_maps.append({
            "embT": embT,
            "embkT": embkT,
            "wqn": wqn,
            "wkn": wkn,
            "wvT": wvT,
            "masks": masks_by_par[par],
        })

    trace = bool(int(os.environ.get("BASS_KERNEL_TRACE", "0")))
    kwargs = {}
    if trace:
        kwargs["trace"] = _install_ntff_hook()

    res = run_bass_kernel_spmd(nc, in_maps, core_ids=list(range(NCORES)),
                               **kwargs)
    _CACHED["last_result"] = res

    out = np.empty((B, S, D), dtype=np.float32)
    for b in range(B):
        r0, r1 = res.results[2 * b], res.results[2 * b + 1]
        av = r0["oav"] + r1["oav"]                      # [S, D]
        l = (r0["ol"] + r1["ol"])                       # [16, 128, 2]
        lfull = l.transpose(0, 2, 1).reshape(S, 1)      # q = 256j+128qs+p
        out[b] = av / lfull
    return out
